# revision 3
# baseline (speedup 1.0000x reference)
"""AttnBlock (C=128, spatial 16x24x24 -> N=9216 tokens, batch 1) on 8 Trainium2
NeuronCores via Bass/Tile.

Strategy (flash-style sequence parallelism, per the sharding hint):
  - The N (token) dim of q is sharded 8 ways: core i handles query tokens
    [i*1152, (i+1)*1152). k/v are "all-gathered" for free: every core receives
    the full x and computes the full k / vT projections locally (0.6 GFLOP of
    duplicated work per core vs ~5.4 GFLOP of attention work).
  - Layout A ("S-transposed"): for each key-chunk kk of 128 tokens the PE
    computes S^T = k_chunk^T @ q (chunk tokens on partitions, q tokens on the
    free axis). ACT applies exp(scale*S^T) straight out of PSUM into bf16 P^T
    tiles; DVE accumulates the softmax denominators (acc += P^T, bf16 2x mode);
    the PE accumulates O^T += vT_chunk^T @ P^T into a persistent PSUM region.
  - Biases: bk cancels in softmax (it shifts every score of a row by the same
    amount) and is dropped. bv commutes with the softmax average (rows sum to
    1) and is added per-partition at the end. bq is applied on the q
    projection. The 1/sqrt(C) scale is folded into the exp activation.
  - End: row-sums via a ones-vector matmul (partition reduction on the PE),
    reciprocal_approx_fast, gpsimd partition-broadcast, normalize + bv + x
    residual, Wp projection, + bp + x again, DMA out.

The full inputs are sharded on the host (pure slicing / dtype casts), each
core runs the same program on its slice, outputs are concatenated.
"""

import sys

for _p in ("/opt/trn_rl_repo",):
    if _p not in sys.path:
        sys.path.append(_p)

import numpy as np
import ml_dtypes

C = 128
Z, HH, WW = 16, 24, 24
N = Z * HH * WW            # 9216 tokens
NCORES = 8
NQ = N // NCORES           # 1152 query tokens per core
CHUNK = 128
NCH = N // CHUNK           # 72 key chunks
SCALE = float(C) ** -0.5
BF16 = ml_dtypes.bfloat16

_CACHE: dict = {}


def _build_nc():
    from contextlib import ExitStack
    import concourse.tile as tile
    from concourse import bacc, mybir

    f32 = mybir.dt.float32
    bf16 = mybir.dt.bfloat16
    AF = mybir.ActivationFunctionType

    nc = bacc.Bacc("TRN2", target_bir_lowering=False, debug=False)

    xb_d = nc.dram_tensor("xb", [C, N], bf16, kind="ExternalInput").ap()
    x32_d = nc.dram_tensor("x32", [C, NQ], f32, kind="ExternalInput").ap()
    yb_d = nc.dram_tensor("yb", [C, NQ], bf16, kind="ExternalInput").ap()
    wq_d = nc.dram_tensor("wqT", [C, C], bf16, kind="ExternalInput").ap()
    wk_d = nc.dram_tensor("wkT", [C, C], bf16, kind="ExternalInput").ap()
    wv_d = nc.dram_tensor("wvT", [C, C], bf16, kind="ExternalInput").ap()
    wp_d = nc.dram_tensor("wpT", [C, C], bf16, kind="ExternalInput").ap()
    bq_d = nc.dram_tensor("bq", [C, 1], f32, kind="ExternalInput").ap()
    bv_d = nc.dram_tensor("bv", [C, 1], f32, kind="ExternalInput").ap()
    bp_d = nc.dram_tensor("bp", [C, 1], f32, kind="ExternalInput").ap()
    out_d = nc.dram_tensor("out", [C, NQ], f32, kind="ExternalOutput").ap()

    # q free-dim split for the three 512-aligned column slices used by
    # matmuls into multi-bank PSUM tiles.
    Q3 = [(0, 512), (512, 512), (1024, 128)]
    # q halves for the two-tensor S^T PSUM scheme (each 576 = 512 + 64).
    HALF = 576

    with tile.TileContext(nc) as tc, ExitStack() as ctx:
        const = ctx.enter_context(tc.tile_pool(name="const", bufs=1))
        big = ctx.enter_context(tc.tile_pool(name="big", bufs=1))
        ptp = ctx.enter_context(tc.tile_pool(name="ptp", bufs=2))
        # PSUM budget: "ps" = 2 slots x (128,576) = 4 banks, "po" = 3 banks.
        ps = ctx.enter_context(tc.tile_pool(name="ps", bufs=2, space="PSUM"))
        po = ctx.enter_context(tc.tile_pool(name="po", bufs=1, space="PSUM"))

        # ---- constants / weights ----
        wq = const.tile([C, C], bf16, tag="wq", name="wq")
        wk = const.tile([C, C], bf16, tag="wk", name="wk")
        wv = const.tile([C, C], bf16, tag="wv", name="wv")
        wp = const.tile([C, C], bf16, tag="wp", name="wp")
        nc.sync.dma_start(wq[:], wq_d)
        nc.sync.dma_start(wk[:], wk_d)
        nc.sync.dma_start(wv[:], wv_d)
        nc.sync.dma_start(wp[:], wp_d)
        bq_t = const.tile([C, 1], f32, tag="bq", name="bq_t")
        bv_t = const.tile([C, 1], f32, tag="bv", name="bv_t")
        bp_t = const.tile([C, 1], f32, tag="bp", name="bp_t")
        nc.sync.dma_start(bq_t[:], bq_d)
        nc.sync.dma_start(bv_t[:], bv_d)
        nc.sync.dma_start(bp_t[:], bp_d)
        ones_col = const.tile([C, 1], bf16, tag="ones", name="ones_col")
        nc.vector.memset(ones_col[:], 1.0)

        # ---- big SBUF residents ----
        xb_sb = big.tile([C, N], bf16, tag="xb_sb", name="xb_sb")
        # split the 2.25MB load so the first projection blocks start early
        for pc in range(6):
            w = N // 6
            nc.sync.dma_start(xb_sb[:, pc * w:(pc + 1) * w], xb_d[:, pc * w:(pc + 1) * w])
        y_sb = big.tile([C, NQ], bf16, tag="y_sb", name="y_sb")
        nc.sync.dma_start(y_sb[:], yb_d)
        x32_sb = big.tile([C, NQ], f32, tag="x32_sb", name="x32_sb")
        nc.sync.dma_start(x32_sb[:], x32_d)
        k_sb = big.tile([C, N], bf16, tag="k_sb", name="k_sb")
        vT_sb = big.tile([C, N], bf16, tag="vT_sb", name="vT_sb")
        q_sb = big.tile([C, NQ], bf16, tag="q_sb", name="q_sb")
        acc = big.tile([C, NQ], bf16, tag="acc", name="acc")
        nc.vector.memset(acc[:], 0.0)

        # ---- q projection: q = Wq^T^T @ y + bq, in (c, nq) layout ----
        for (c0, w) in Q3:
            qp = ps.tile([C, HALF], f32, tag="s", name=f"qp_{c0}")
            nc.tensor.matmul(qp[:, :w], wq[:], y_sb[:, c0:c0 + w], start=True, stop=True)
            nc.scalar.activation(q_sb[:, c0:c0 + w], qp[:, :w], AF.Identity, bias=bq_t[:])

        # ---- O^T accumulator (c on partitions, q on free axis) ----
        o_acc = po.tile([C, NQ], f32, tag="o_acc", name="o_acc")

        # ---- main loop over 72 key chunks ----
        for j in range(NCH):
            if j % 4 == 0:
                b = j // 4
                # k projection for key block b (512 keys), (c, n) layout
                kp = ps.tile([C, HALF], f32, tag="s", name=f"kp_{b}")
                nc.tensor.matmul(kp[:, :512], wk[:], xb_sb[:, b * 512:(b + 1) * 512],
                                 start=True, stop=True)
                nc.vector.tensor_copy(k_sb[:, b * 512:(b + 1) * 512], kp[:, :512])
                # vT projection for chunks 4b..4b+3, (n_local, c) layout
                vp = ps.tile([C, HALF], f32, tag="s", name=f"vp_{b}")
                for i in range(4):
                    ch = 4 * b + i
                    nc.tensor.matmul(vp[:, i * 128:(i + 1) * 128],
                                     xb_sb[:, ch * 128:(ch + 1) * 128], wv[:],
                                     start=(i == 0), stop=(i == 3))
                nc.vector.tensor_copy(vT_sb[:, b * 512:(b + 1) * 512], vp[:, :512])

            kch = k_sb[:, j * 128:(j + 1) * 128]
            pt = ptp.tile([C, NQ], bf16, tag="pt", name=f"pt_{j}")
            for h0 in (0, HALF):
                sh = ps.tile([C, HALF], f32, tag="s", name=f"s_{j}_{h0}")
                nc.tensor.matmul(sh[:, :512], kch, q_sb[:, h0:h0 + 512],
                                 start=True, stop=True)
                nc.tensor.matmul(sh[:, 512:HALF], kch, q_sb[:, h0 + 512:h0 + HALF],
                                 start=True, stop=True)
                nc.scalar.activation(pt[:, h0:h0 + HALF], sh[:, :HALF], AF.Exp,
                                     scale=SCALE)
            nc.vector.tensor_add(acc[:], acc[:], pt[:])
            vch = vT_sb[:, j * 128:(j + 1) * 128]
            for (c0, w) in Q3:
                nc.tensor.matmul(o_acc[:, c0:c0 + w], vch, pt[:, c0:c0 + w],
                                 start=(j == 0), stop=(j == NCH - 1),
                                 skip_group_check=True)

        # ---- softmax denominators: rowsum over all keys, then 1/r ----
        rs_row = big.tile([1, NQ], f32, tag="rs_row", name="rs_row")
        for (c0, w) in Q3:
            rp = ps.tile([C, HALF], f32, tag="s", name=f"rp_{c0}")
            nc.tensor.matmul(rp[:1, :w], ones_col[:], acc[:, c0:c0 + w],
                             start=True, stop=True)
            nc.scalar.activation(rs_row[:, c0:c0 + w], rp[:1, :w], AF.Copy)
        recip = big.tile([1, NQ], f32, tag="recip", name="recip")
        nc.vector.reciprocal_approx_fast(out=recip[:], in_=rs_row[:])
        rb = big.tile([C, NQ], f32, tag="rb", name="rb")
        nc.gpsimd.partition_broadcast(rb[:], recip[:])

        # ---- h = O/r + bv + x  (first residual), cast bf16 for the Wp GEMM ----
        t1 = big.tile([C, NQ], f32, tag="t1", name="t1")
        nc.vector.tensor_mul(t1[:], o_acc[:], rb[:])
        nc.vector.tensor_scalar_add(t1[:], t1[:], bv_t[:])
        h_bf = big.tile([C, NQ], bf16, tag="h_bf", name="h_bf")
        nc.vector.tensor_add(h_bf[:], t1[:], x32_sb[:])

        # ---- out = Wp h + bp + x  (second residual) ----
        out_sb = big.tile([C, NQ], f32, tag="out_sb", name="out_sb")
        for (c0, w) in Q3:
            pp = ps.tile([C, HALF], f32, tag="s", name=f"pp_{c0}")
            nc.tensor.matmul(pp[:, :w], wp[:], h_bf[:, c0:c0 + w], start=True, stop=True)
            nc.scalar.activation(out_sb[:, c0:c0 + w], pp[:, :w], AF.Identity,
                                 bias=bp_t[:])
        nc.vector.tensor_add(out_sb[:], out_sb[:], x32_sb[:])
        nc.sync.dma_start(out_d, out_sb[:])

    nc.compile()
    return nc


def make_in_maps(x, y, Wq, bq, Wk, bk, Wv, bv, Wp, bp):
    """Host-side sharding: slice q/residual tokens per core, cast matmul
    operands to bf16, pre-transpose the 1x1-conv weights into lhsT layout."""
    x2 = np.asarray(x, np.float32).reshape(C, N)
    y2 = np.asarray(y, np.float32).reshape(C, N)
    xb = np.ascontiguousarray(x2).astype(BF16)
    wqT = np.ascontiguousarray(np.asarray(Wq, np.float32).T).astype(BF16)
    wkT = np.ascontiguousarray(np.asarray(Wk, np.float32).T).astype(BF16)
    wvT = np.ascontiguousarray(np.asarray(Wv, np.float32).T).astype(BF16)
    wpT = np.ascontiguousarray(np.asarray(Wp, np.float32).T).astype(BF16)
    bq2 = np.asarray(bq, np.float32).reshape(C, 1)
    bv2 = np.asarray(bv, np.float32).reshape(C, 1)
    bp2 = np.asarray(bp, np.float32).reshape(C, 1)
    in_maps = []
    for i in range(NCORES):
        sl = slice(i * NQ, (i + 1) * NQ)
        in_maps.append({
            "xb": xb,
            "x32": np.ascontiguousarray(x2[:, sl]),
            "yb": np.ascontiguousarray(y2[:, sl]).astype(BF16),
            "wqT": wqT, "wkT": wkT, "wvT": wvT, "wpT": wpT,
            "bq": bq2, "bv": bv2, "bp": bp2,
        })
    return in_maps


class Runner:
    """Compiles the SPMD program once and exposes a repeat-callable runner
    (mirrors concourse.bass2jax.run_bass_via_pjrt's multi-core path, but
    caches the jitted executable so repeat calls don't recompile)."""

    def __init__(self):
        import jax
        try:
            jax.config.update("jax_compilation_cache_dir", "/tmp/jax_neff_cache")
            jax.config.update("jax_persistent_cache_min_compile_time_secs", 1.0)
        except Exception:
            pass
        from jax.sharding import Mesh, PartitionSpec, NamedSharding
        from jax.experimental.shard_map import shard_map
        from concourse import mybir
        from concourse import bass2jax

        bass2jax.install_neuronx_cc_hook()
        nc = _build_nc()
        self.nc = nc
        self.jax = jax

        partition_name = nc.partition_id_tensor.name if nc.partition_id_tensor else None
        in_names, out_names, out_avals, zero_templates = [], [], [], []
        for alloc in nc.m.functions[0].allocations:
            if not isinstance(alloc, mybir.MemoryLocationSet):
                continue
            name = alloc.memorylocations[0].name
            if alloc.kind == "ExternalInput":
                if name != partition_name:
                    in_names.append(name)
            elif alloc.kind == "ExternalOutput":
                out_names.append(name)
                shape = tuple(alloc.tensor_shape)
                dtype = mybir.dt.np(alloc.dtype)
                out_avals.append(jax.core.ShapedArray(shape, dtype))
                zero_templates.append(np.zeros(shape, dtype))
        self.in_names, self.out_names = in_names, out_names
        self.out_avals, self.zero_templates = out_avals, zero_templates
        n_params = len(in_names)
        self.n_params = n_params
        all_in_names = tuple(in_names) + tuple(out_names)
        if partition_name is not None:
            all_in_names = all_in_names + (partition_name,)

        def _body(*args):
            operands = list(args)
            if partition_name is not None:
                operands.append(bass2jax.partition_id_tensor())
            outs = bass2jax._bass_exec_p.bind(
                *operands,
                out_avals=tuple(out_avals),
                in_names=all_in_names,
                out_names=tuple(out_names),
                lowering_input_output_aliases=(),
                sim_require_finite=True,
                sim_require_nnan=True,
                nc=nc,
            )
            return tuple(outs)

        devices = jax.devices()[:NCORES]
        assert len(devices) == NCORES, f"need {NCORES} cores, got {len(devices)}"
        self.mesh = Mesh(np.asarray(devices), ("core",))
        self.spec = PartitionSpec("core")
        self.sharding = NamedSharding(self.mesh, self.spec)
        n_outs = len(out_names)
        in_specs = (self.spec,) * (n_params + n_outs)
        out_specs = (self.spec,) * n_outs
        # no donation: lets us reuse staged device buffers across timed calls
        self.sharded = jax.jit(
            shard_map(_body, mesh=self.mesh, in_specs=in_specs,
                      out_specs=out_specs, check_rep=False),
            keep_unused=True,
        )

    def stage(self, in_maps):
        """device_put the concatenated per-core inputs (+ zero out-buffers)."""
        jax = self.jax
        concat = [
            np.concatenate([np.asarray(in_maps[c][nm]) for c in range(NCORES)], axis=0)
            for nm in self.in_names
        ]
        concat += [
            np.zeros((NCORES * z.shape[0],) + z.shape[1:], z.dtype)
            for z in self.zero_templates
        ]
        return [jax.device_put(a, self.sharding) for a in concat]

    def run_staged(self, staged):
        return self.sharded(*staged)

    def __call__(self, in_maps):
        jax = self.jax
        out_arrs = self.sharded(*self.stage(in_maps))
        out_arrs = [np.asarray(a) for a in jax.block_until_ready(out_arrs)]
        results = []
        for c in range(NCORES):
            results.append({
                nm: out_arrs[i].reshape(NCORES, *self.out_avals[i].shape)[c]
                for i, nm in enumerate(self.out_names)
            })
        return results


def get_runner():
    if "runner" not in _CACHE:
        _CACHE["runner"] = Runner()
    return _CACHE["runner"]


def kernel(**inputs) -> np.ndarray:
    runner = get_runner()
    in_maps = make_in_maps(**{k: inputs[k] for k in
                              ("x", "y", "Wq", "bq", "Wk", "bk", "Wv", "bv", "Wp", "bp")})
    results = runner(in_maps)
    out = np.concatenate([results[i]["out"] for i in range(NCORES)], axis=1)
    return out.reshape(1, C, Z, HH, WW).astype(np.float32)


# revision 38
# speedup vs baseline: 29.7127x; 29.7127x over previous
"""AttnBlock (C=128, spatial 16x24x24 -> N=9216 tokens, batch 1) on 8 Trainium2
NeuronCores via Bass/Tile.

Strategy (flash-style sequence parallelism, per the sharding hint):
  - The N (token) dim of q is sharded 8 ways: core i handles query tokens
    [i*1152, (i+1)*1152). k/v are "all-gathered" for free: every core receives
    the full x and computes the full k / vT projections locally.
  - Layout A ("S-transposed"): for each key-chunk kk of 128 tokens the PE
    computes S^T = k_chunk^T @ q (chunk tokens on partitions, q tokens on the
    free axis). ACT applies exp(scale*S^T) straight out of PSUM into bf16 P^T
    tiles; DVE (+ gpsimd for 1-in-8 chunks) accumulates softmax denominators;
    the PE accumulates O^T += vT_chunk^T @ P^T into a persistent PSUM region.
  - PSUM layout (8 banks): sA = (128,1536) [3 banks] holds even chunks'
    S^T (cols 0:1152) plus odd chunks' last-128 S^T columns (cols 1280:1408,
    i.e. the bank-2 spare, written only after the even chunk's exp consumed
    bank 2); sB = (128,1024) [2 banks] holds odd chunks' first 1024 columns.
    This lets even chunks run ONE 1152-wide exp (measured 1092ns vs 2x736ns)
    and odd chunks a 1024-wide + 128-wide pair.  o_acc = (128,1152) [3 banks].
    The q/k/vT projections stream through a separate 3-slot pool that LIVES IN
    o_acc's banks: the projection pool closes before o_acc opens, and the
    first CATCH chunks' O-matmuls are deferred (their P^T tiles are buffered
    in SBUF) until the projections vacate PSUM, then the PE catches up at two
    O-chunks per new chunk.
  - Biases: bk cancels in softmax and is dropped; bv commutes with the
    softmax average; bq is applied on the q projection; 1/sqrt(C) is folded
    into the exp.  The output is assembled as out = (Wp O)/r + g where
    g = Wp x + x + (Wp bv + bp) is computed mid-loop off the critical path.

The full inputs are sharded on the host (pure slicing / dtype casts), each
core runs the same program on its slice, outputs are concatenated.
"""

import sys

for _p in ("/opt/trn_rl_repo",):
    if _p not in sys.path:
        sys.path.append(_p)

import numpy as np
import ml_dtypes

C = 128
Z, HH, WW = 16, 24, 24
N = Z * HH * WW            # 9216 tokens
NCORES = 8
NQ = N // NCORES           # 1152 query tokens per core
CHUNK = 128
NCH = N // CHUNK           # 72 key chunks
SCALE = float(C) ** -0.5
BF16 = ml_dtypes.bfloat16
CATCH = 12                 # chunks whose O-matmuls are deferred


def _build_nc(repeat: int = 1):
    from contextlib import ExitStack
    import concourse.tile as tile
    from concourse import bacc, mybir

    f32 = mybir.dt.float32
    bf16 = mybir.dt.bfloat16
    AF = mybir.ActivationFunctionType
    ADD = mybir.AluOpType.add

    nc = bacc.Bacc("TRN2", target_bir_lowering=False, debug=False)

    xb_d = nc.dram_tensor("xb", [C, N], bf16, kind="ExternalInput").ap()
    x32_d = nc.dram_tensor("x32", [C, NQ], f32, kind="ExternalInput").ap()
    yb_d = nc.dram_tensor("yb", [C, NQ], bf16, kind="ExternalInput").ap()
    # packed [WqT | WkT | WvT | WpT] and [bq | bv | bp] (fewer DMA issues)
    wcat_d = nc.dram_tensor("wcat", [C, 4 * C], bf16, kind="ExternalInput").ap()
    bcat_d = nc.dram_tensor("bcat", [C, 3], f32, kind="ExternalInput").ap()
    out_d = nc.dram_tensor("out", [C, NQ], f32, kind="ExternalOutput").ap()

    Q3 = [(0, 512), (512, 512), (1024, 128)]
    HALF = 576

    with tile.TileContext(nc) as tc, ExitStack() as ctx:
        const = ctx.enter_context(tc.tile_pool(name="const", bufs=1))
        big = ctx.enter_context(tc.tile_pool(name="big", bufs=1))
        ptp = ctx.enter_context(tc.tile_pool(name="ptp", bufs=CATCH + 4))

        # ---- constants / weights (loaded once) ----
        wcat = const.tile([C, 4 * C], bf16, tag="wcat", name="wcat")
        nc.sync.dma_start(wcat[:], wcat_d)
        wq, wk, wv, wp = (wcat[:, i * C:(i + 1) * C] for i in range(4))
        bcat = const.tile([C, 3], f32, tag="bcat", name="bcat")
        nc.sync.dma_start(bcat[:], bcat_d)
        bq_t, bv_t, bp_t = (bcat[:, i:i + 1] for i in range(3))
        ones_col = const.tile([C, 1], bf16, tag="ones", name="ones_col")
        nc.vector.memset(ones_col[:], 1.0)

        def emit_compute():
            # ---- big SBUF residents ----
            y_sb = big.tile([C, NQ], bf16, tag="y_sb", name="y_sb")
            nc.sync.dma_start(y_sb[:], yb_d)
            xb_sb = big.tile([C, N], bf16, tag="xb_sb", name="xb_sb")
            for pc in range(6):
                w = N // 6
                nc.sync.dma_start(xb_sb[:, pc * w:(pc + 1) * w],
                                  xb_d[:, pc * w:(pc + 1) * w])
            x32_sb = big.tile([C, NQ], f32, tag="x32_sb", name="x32_sb")
            nc.sync.dma_start(x32_sb[:], x32_d)
            k_sb = big.tile([C, N], bf16, tag="k_sb", name="k_sb")
            vT_sb = big.tile([C, N], bf16, tag="vT_sb", name="vT_sb")
            q_sb = big.tile([C, NQ], bf16, tag="q_sb", name="q_sb")
            # softmax-denominator accumulators: main (q cols 0:1024) on DVE
            # with 1-in-8 chunks on gpsimd; the batched tails separately.
            acc = big.tile([C, 1024], bf16, tag="acc", name="acc")
            nc.vector.memset(acc[:], 0.0)
            acc2 = big.tile([C, 1024], bf16, tag="acc2", name="acc2")
            nc.gpsimd.memset(acc2[:], 0.0)
            acc_t = big.tile([C, 512], bf16, tag="acc_t", name="acc_t")
            nc.vector.memset(acc_t[:], 0.0)

            # persistent S^T psum tensors: sA/sB (128,1024) ping-pong [2+2
            # banks] for q cols 0:1024, sT (128,512) [1 bank] stages the last
            # 128 q-columns of four consecutive chunks for one batched exp.
            # RIGHT side so the early release is independent of the left-side
            # pj2 -> po -> ptail stack.
            psS = tc.alloc_tile_pool(name="psS", bufs=1, space="PSUM",
                                     side="right")
            sA = psS.tile([C, 1024], f32, tag="sA", name="sA")
            sB = psS.tile([C, 1024], f32, tag="sB", name="sB")
            sT = psS.tile([C, 512], f32, tag="sT", name="sT")

            # g = Wp x + x + (Wp bv + bp), emitted mid-phase-1 through pj2
            gtiles = {}

            def emit_g_term(pool):
                xq_bf = big.tile([C, NQ], bf16, tag="xq_bf", name="xq_bf")
                nc.vector.tensor_copy(xq_bf[:], x32_sb[:])
                bv_bf = big.tile([C, 1], bf16, tag="bv_bf", name="bv_bf")
                nc.vector.tensor_copy(bv_bf[:], bv_t[:])
                gb_ps = pool.tile([C, 512], f32, tag="pj2", name="gb_ps")
                nc.tensor.matmul(gb_ps[:, :1], wp[:], bv_bf[:], start=True, stop=True)
                gb = big.tile([C, 1], f32, tag="gb", name="gb")
                nc.vector.tensor_scalar_add(gb[:], gb_ps[:, :1], bp_t[:])
                g = big.tile([C, NQ], f32, tag="g", name="g")
                for (c0, w) in Q3:
                    gp = pool.tile([C, 512], f32, tag="pj2", name=f"gp_{c0}")
                    nc.tensor.matmul(gp[:, :w], wp[:], xq_bf[:, c0:c0 + w],
                                     start=True, stop=True)
                    nc.vector.scalar_tensor_tensor(
                        g[:, c0:c0 + w], gp[:, :w], gb[:],
                        x32_sb[:, c0:c0 + w], op0=ADD, op1=ADD)
                gtiles["g"] = g

            pt_tiles = {}
            ptt_tiles = {}

            def emit_s_exp(j):
                """S^T matmuls + exp + denominator accumulation for chunk j.
                q cols 0:1024 take the wide path through sA/sB; the last 128
                columns stage in sT and exp once per 4 chunks."""
                kch = k_sb[:, j * 128:(j + 1) * 128]
                pt = ptp.tile([C, 1024], bf16, tag="pt", name=f"pt_{j}")
                pt_tiles[j] = pt
                slot = sA if j % 2 == 0 else sB
                nc.tensor.matmul(slot[:, 0:512], kch, q_sb[:, 0:512],
                                 start=True, stop=True)
                nc.tensor.matmul(slot[:, 512:1024], kch, q_sb[:, 512:1024],
                                 start=True, stop=True)
                r = j % 4
                nc.tensor.matmul(sT[:, r * 128:(r + 1) * 128], kch,
                                 q_sb[:, 1024:1152],
                                 start=(r == 0), stop=(r == 3),
                                 skip_group_check=True)
                nc.scalar.activation(pt[:, :1024], slot[:, :1024], AF.Exp,
                                     scale=SCALE)
                if j % 8 == 3:
                    nc.gpsimd.tensor_add(acc2[:], acc2[:], pt[:])
                else:
                    nc.vector.tensor_add(acc[:], acc[:], pt[:])
                if r == 3:
                    g = j // 4
                    ptt = ptt_tiles[g] = ptp.tile([C, 512], bf16, tag="ptt",
                                                  name=f"ptt_{g}", bufs=6)
                    nc.scalar.activation(ptt[:, :512], sT[:, :512], AF.Exp,
                                         scale=SCALE)
                    nc.vector.tensor_add(acc_t[:], acc_t[:], ptt[:])

            def emit_o(j, o_acc):
                """Deferred-able stage-2 accumulation for chunk j (needs the
                chunk's 4-group tail exp done, i.e. chunk 4*(j//4)+3)."""
                vch = vT_sb[:, j * 128:(j + 1) * 128]
                pt = pt_tiles.pop(j)
                nc.tensor.matmul(o_acc[:, 0:512], vch, pt[:, 0:512],
                                 start=(j == 0), stop=(j == NCH - 1),
                                 skip_group_check=True)
                nc.tensor.matmul(o_acc[:, 512:1024], vch, pt[:, 512:1024],
                                 start=(j == 0), stop=(j == NCH - 1),
                                 skip_group_check=True)
                g, r = j // 4, j % 4
                ptt = ptt_tiles[g]
                nc.tensor.matmul(o_acc[:, 1024:1152], vch,
                                 ptt[:, r * 128:(r + 1) * 128],
                                 start=(j == 0), stop=(j == NCH - 1),
                                 skip_group_check=True)

            # ---- phase 1: projections (PSUM pool in o_acc's future banks)
            # interleaved with the first CATCH chunks' S/exp work ----
            pj2 = tc.alloc_tile_pool(name="pj2", bufs=3, space="PSUM")
            # q projection first (gates everything)
            for (c0, w) in Q3:
                qp = pj2.tile([C, 512], f32, tag="pj2", name=f"qp_{c0}")
                nc.tensor.matmul(qp[:, :w], wq[:], y_sb[:, c0:c0 + w],
                                 start=True, stop=True)
                nc.scalar.activation(q_sb[:, c0:c0 + w], qp[:, :w],
                                     AF.Identity, bias=bq_t[:])

            def emit_proj_block(b):
                sl = slice(b * 512, (b + 1) * 512)
                kp = pj2.tile([C, 512], f32, tag="pj2", name=f"kp_{b}")
                nc.tensor.matmul(kp[:, :512], wk[:], xb_sb[:, sl],
                                 start=True, stop=True)
                nc.vector.tensor_copy(k_sb[:, sl], kp[:, :512])
                # vT in (n, c) layout: per-chunk GEMMs with the x-chunk
                # stationary (the transpose comes out of the matmul itself)
                vp = pj2.tile([C, 512], f32, tag="pj2", name=f"vp_{b}")
                for i in range(4):
                    ch = 4 * b + i
                    nc.tensor.matmul(vp[:, i * 128:(i + 1) * 128],
                                     xb_sb[:, ch * 128:(ch + 1) * 128], wv[:],
                                     start=(i == 0), stop=(i == 3))
                nc.vector.tensor_copy(vT_sb[:, b * 512:(b + 1) * 512],
                                      vp[:, :512])

            emit_proj_block(0)
            for j in range(CATCH):
                # two more projection blocks per chunk until all 18 done
                for b in (2 * j + 1, 2 * j + 2):
                    if b < NCH // 4:
                        emit_proj_block(b)
                if j == 9:
                    emit_g_term(pj2)
                emit_s_exp(j)
            pj2.release()

            # ---- phase 2: o_acc opens in the freed banks; catch up at two
            # deferred O-chunks per new chunk, then run 1:1 ----
            po = tc.alloc_tile_pool(name="po", bufs=1, space="PSUM")
            o_acc = po.tile([C, NQ], f32, tag="o_acc", name="o_acc")
            next_o = 0
            for j in range(CATCH, NCH):
                emit_s_exp(j)
                budget = 2
                while budget > 0 and next_o <= j - 3:
                    emit_o(next_o, o_acc)
                    next_o += 1
                    budget -= 1
            while next_o < NCH:
                emit_o(next_o, o_acc)
                next_o += 1

            # ---- tail:  out = (Wp O)/r + g.
            # sA/sB/sT are dead now; their banks host the tail psum pool. ----
            psS.release()
            ptail = tc.alloc_tile_pool(name="ptail", bufs=1, space="PSUM")
            g = gtiles["g"]
            o_bf = big.tile([C, NQ], bf16, tag="o_bf", name="o_bf")
            rs_row = big.tile([1, NQ], f32, tag="rs_row", name="rs_row")
            recip = big.tile([1, NQ], f32, tag="recip", name="recip")
            rb = big.tile([C, NQ], f32, tag="rb", name="rb")
            t2 = big.tile([C, NQ], f32, tag="t2", name="t2")
            out_sb = big.tile([C, NQ], f32, tag="out_sb", name="out_sb")
            # denominators: main part from acc+acc2, tails folded from acc_t
            rpA = ptail.tile([C, 1024], f32, tag="rp", name="rpA")
            for c0 in (0, 512):
                nc.tensor.matmul(rpA[:1, c0:c0 + 512], ones_col[:],
                                 acc[:, c0:c0 + 512], start=True, stop=False)
                nc.tensor.matmul(rpA[:1, c0:c0 + 512], ones_col[:],
                                 acc2[:, c0:c0 + 512], start=False, stop=True)
            nc.vector.tensor_copy(rs_row[:, 0:1024], rpA[:1, :1024])
            rpB = ptail.tile([C, 512], f32, tag="rpt", name="rpB")
            nc.tensor.matmul(rpB[:1, :512], ones_col[:], acc_t[:, :512],
                             start=True, stop=True)
            tsb = big.tile([1, 512], f32, tag="tsb", name="tsb")
            nc.vector.tensor_copy(tsb[:], rpB[:1, :512])
            nc.vector.tensor_add(rs_row[:, 1024:1152], tsb[:, 0:128],
                                 tsb[:, 128:256])
            nc.vector.tensor_add(rs_row[:, 1024:1152], rs_row[:, 1024:1152],
                                 tsb[:, 256:384])
            nc.vector.tensor_add(rs_row[:, 1024:1152], rs_row[:, 1024:1152],
                                 tsb[:, 384:512])
            nc.vector.reciprocal_approx_fast(out=recip[:], in_=rs_row[:])
            nc.gpsimd.partition_broadcast(rb[:], recip[:])
            # O evac + projection + normalize, per q-half
            for h0 in (0, HALF):
                nc.vector.tensor_copy(o_bf[:, h0:h0 + HALF],
                                      o_acc[:, h0:h0 + HALF])
                pw = ptail.tile([C, HALF], f32, tag="pw", name=f"pw_{h0}")
                nc.tensor.matmul(pw[:, :512], wp[:], o_bf[:, h0:h0 + 512],
                                 start=True, stop=True)
                nc.tensor.matmul(pw[:, 512:HALF], wp[:],
                                 o_bf[:, h0 + 512:h0 + HALF],
                                 start=True, stop=True)
                nc.vector.tensor_mul(t2[:, h0:h0 + HALF], pw[:, :HALF],
                                     rb[:, h0:h0 + HALF])
                nc.vector.tensor_add(out_sb[:, h0:h0 + HALF],
                                     t2[:, h0:h0 + HALF], g[:, h0:h0 + HALF])
                nc.sync.dma_start(out_d[:, h0:h0 + HALF],
                                  out_sb[:, h0:h0 + HALF])
            ptail.release()
            po.release()

        for _rep in range(repeat):
            emit_compute()

    nc.compile()
    return nc


def make_in_maps(x, y, Wq, bq, Wk, bk, Wv, bv, Wp, bp):
    """Host-side sharding: slice q/residual tokens per core, cast matmul
    operands to bf16, pre-transpose the 1x1-conv weights into lhsT layout."""
    x2 = np.asarray(x, np.float32).reshape(C, N)
    y2 = np.asarray(y, np.float32).reshape(C, N)
    xb = np.ascontiguousarray(x2).astype(BF16)
    wcat = np.ascontiguousarray(np.concatenate(
        [np.asarray(w, np.float32).T for w in (Wq, Wk, Wv, Wp)], axis=1)).astype(BF16)
    bcat = np.ascontiguousarray(np.stack(
        [np.asarray(b, np.float32) for b in (bq, bv, bp)], axis=1))
    in_maps = []
    for i in range(NCORES):
        sl = slice(i * NQ, (i + 1) * NQ)
        in_maps.append({
            "xb": xb,
            "x32": np.ascontiguousarray(x2[:, sl]),
            "yb": np.ascontiguousarray(y2[:, sl]).astype(BF16),
            "wcat": wcat, "bcat": bcat,
        })
    return in_maps


_CACHE: dict = {}


class Runner:
    """Compiles the SPMD program once and exposes a repeat-callable runner
    (mirrors concourse.bass2jax.run_bass_via_pjrt's multi-core path, but
    caches the jitted executable so repeat calls don't recompile)."""

    def __init__(self, repeat: int = 1):
        import jax
        try:
            jax.config.update("jax_compilation_cache_dir", "/tmp/jax_neff_cache")
            jax.config.update("jax_persistent_cache_min_compile_time_secs", 1.0)
        except Exception:
            pass
        from jax.sharding import Mesh, PartitionSpec, NamedSharding
        from jax.experimental.shard_map import shard_map
        from concourse import mybir
        from concourse import bass2jax

        bass2jax.install_neuronx_cc_hook()
        nc = _build_nc(repeat=repeat)
        self.nc = nc
        self.jax = jax

        partition_name = nc.partition_id_tensor.name if nc.partition_id_tensor else None
        in_names, out_names, out_avals, zero_templates = [], [], [], []
        for alloc in nc.m.functions[0].allocations:
            if not isinstance(alloc, mybir.MemoryLocationSet):
                continue
            name = alloc.memorylocations[0].name
            if alloc.kind == "ExternalInput":
                if name != partition_name:
                    in_names.append(name)
            elif alloc.kind == "ExternalOutput":
                out_names.append(name)
                shape = tuple(alloc.tensor_shape)
                dtype = mybir.dt.np(alloc.dtype)
                out_avals.append(jax.core.ShapedArray(shape, dtype))
                zero_templates.append(np.zeros(shape, dtype))
        self.in_names, self.out_names = in_names, out_names
        self.out_avals, self.zero_templates = out_avals, zero_templates
        n_params = len(in_names)
        self.n_params = n_params
        all_in_names = tuple(in_names) + tuple(out_names)
        if partition_name is not None:
            all_in_names = all_in_names + (partition_name,)

        def _body(*args):
            operands = list(args)
            if partition_name is not None:
                operands.append(bass2jax.partition_id_tensor())
            outs = bass2jax._bass_exec_p.bind(
                *operands,
                out_avals=tuple(out_avals),
                in_names=all_in_names,
                out_names=tuple(out_names),
                lowering_input_output_aliases=(),
                sim_require_finite=True,
                sim_require_nnan=True,
                nc=nc,
            )
            return tuple(outs)

        devices = jax.devices()[:NCORES]
        assert len(devices) == NCORES, f"need {NCORES} cores, got {len(devices)}"
        self.mesh = Mesh(np.asarray(devices), ("core",))
        self.spec = PartitionSpec("core")
        self.sharding = NamedSharding(self.mesh, self.spec)
        n_outs = len(out_names)
        in_specs = (self.spec,) * (n_params + n_outs)
        out_specs = (self.spec,) * n_outs
        # no donation: lets us reuse staged device buffers across timed calls
        self.sharded = jax.jit(
            shard_map(_body, mesh=self.mesh, in_specs=in_specs,
                      out_specs=out_specs, check_rep=False),
            keep_unused=True,
        )

    def stage(self, in_maps):
        """device_put the concatenated per-core inputs (+ zero out-buffers)."""
        jax = self.jax
        concat = [
            np.concatenate([np.asarray(in_maps[c][nm]) for c in range(NCORES)], axis=0)
            for nm in self.in_names
        ]
        concat += [
            np.zeros((NCORES * z.shape[0],) + z.shape[1:], z.dtype)
            for z in self.zero_templates
        ]
        return [jax.device_put(a, self.sharding) for a in concat]

    def run_staged(self, staged):
        return self.sharded(*staged)

    def __call__(self, in_maps):
        jax = self.jax
        out_arrs = self.sharded(*self.stage(in_maps))
        out_arrs = [np.asarray(a) for a in jax.block_until_ready(out_arrs)]
        results = []
        for c in range(NCORES):
            results.append({
                nm: out_arrs[i].reshape(NCORES, *self.out_avals[i].shape)[c]
                for i, nm in enumerate(self.out_names)
            })
        return results


def get_runner(repeat: int = 1):
    key = ("runner", repeat)
    if key not in _CACHE:
        _CACHE[key] = Runner(repeat=repeat)
    return _CACHE[key]


def kernel(**inputs) -> np.ndarray:
    runner = get_runner()
    in_maps = make_in_maps(**{k: inputs[k] for k in
                              ("x", "y", "Wq", "bq", "Wk", "bk", "Wv", "bv", "Wp", "bp")})
    results = runner(in_maps)
    out = np.concatenate([results[i]["out"] for i in range(NCORES)], axis=1)
    return out.reshape(1, C, Z, HH, WW).astype(np.float32)


# revision 46
# speedup vs baseline: 33.6660x; 1.1331x over previous
"""AttnBlock (C=128, spatial 16x24x24 -> N=9216 tokens, batch 1) on 8 Trainium2
NeuronCores via Bass/Tile.

Strategy (flash-style sequence parallelism, per the sharding hint):
  - The N (token) dim of q is sharded 8 ways: core i handles query tokens
    [i*1152, (i+1)*1152). k/v are "all-gathered" for free: every core receives
    the full x and computes the full k / vT projections locally.
  - Layout A ("S-transposed"): for each key-chunk kk of 128 tokens the PE
    computes S^T = k_chunk^T @ q (chunk tokens on partitions, q tokens on the
    free axis). ACT applies exp(scale*S^T) straight out of PSUM into bf16 P^T
    tiles; DVE (+ gpsimd for 1-in-8 chunks) accumulates softmax denominators;
    the PE accumulates O^T += vT_chunk^T @ P^T into a persistent PSUM region.
  - PSUM layout (8 banks): sA = (128,1536) [3 banks] holds even chunks'
    S^T (cols 0:1152) plus odd chunks' last-128 S^T columns (cols 1280:1408,
    i.e. the bank-2 spare, written only after the even chunk's exp consumed
    bank 2); sB = (128,1024) [2 banks] holds odd chunks' first 1024 columns.
    This lets even chunks run ONE 1152-wide exp (measured 1092ns vs 2x736ns)
    and odd chunks a 1024-wide + 128-wide pair.  o_acc = (128,1152) [3 banks].
    The q/k/vT projections stream through a separate 3-slot pool that LIVES IN
    o_acc's banks: the projection pool closes before o_acc opens, and the
    first CATCH chunks' O-matmuls are deferred (their P^T tiles are buffered
    in SBUF) until the projections vacate PSUM, then the PE catches up at two
    O-chunks per new chunk.
  - Biases: bk cancels in softmax and is dropped; bv commutes with the
    softmax average; bq is applied on the q projection; 1/sqrt(C) is folded
    into the exp.  The output is assembled as out = (Wp O)/r + g where
    g = Wp x + x + (Wp bv + bp) is computed mid-loop off the critical path.

The full inputs are sharded on the host (pure slicing / dtype casts), each
core runs the same program on its slice, outputs are concatenated.
"""

import sys

for _p in ("/opt/trn_rl_repo",):
    if _p not in sys.path:
        sys.path.append(_p)

import numpy as np
import ml_dtypes

C = 128
Z, HH, WW = 16, 24, 24
N = Z * HH * WW            # 9216 tokens
NCORES = 8
NQ = N // NCORES           # 1152 query tokens per core
CHUNK = 128
NCH = N // CHUNK           # 72 key chunks
SCALE = float(C) ** -0.5
BF16 = ml_dtypes.bfloat16
CATCH = 12                 # chunks whose O-matmuls are deferred


def _build_nc(repeat: int = 1):
    from contextlib import ExitStack
    import concourse.tile as tile
    from concourse import bacc, mybir

    f32 = mybir.dt.float32
    bf16 = mybir.dt.bfloat16
    AF = mybir.ActivationFunctionType
    ADD = mybir.AluOpType.add

    nc = bacc.Bacc("TRN2", target_bir_lowering=False, debug=False)

    xb_d = nc.dram_tensor("xb", [C, N], bf16, kind="ExternalInput").ap()
    xbT_d = nc.dram_tensor("xbT", [C, N], bf16, kind="ExternalInput").ap()
    x32_d = nc.dram_tensor("x32", [C, NQ], f32, kind="ExternalInput").ap()
    yb_d = nc.dram_tensor("yb", [C, NQ], bf16, kind="ExternalInput").ap()
    # packed [Wq | Wk | Wv | WpT] and [bq | bv | bp] (fewer DMA issues).
    # Wq/Wk/Wv ship UNtransposed: the kernel never materializes k or v --
    #   S^T = (Wk x)^T q = x^T (Wk^T q)  with  qk := (Wq^T Wk)^T y + Wk^T bq
    #   O   = Wv M,  M := sum_n x[:,n] P^T[n,:],  out_attn = (Wp Wv) M / r
    wcat_d = nc.dram_tensor("wcat", [C, 4 * C], bf16, kind="ExternalInput").ap()
    bcat_d = nc.dram_tensor("bcat", [C, 3], f32, kind="ExternalInput").ap()
    out_d = nc.dram_tensor("out", [C, NQ], f32, kind="ExternalOutput").ap()

    Q3 = [(0, 512), (512, 512), (1024, 128)]
    HALF = 576

    with tile.TileContext(nc) as tc, ExitStack() as ctx:
        const = ctx.enter_context(tc.tile_pool(name="const", bufs=1))
        big = ctx.enter_context(tc.tile_pool(name="big", bufs=1))
        ptp = ctx.enter_context(tc.tile_pool(name="ptp", bufs=CATCH + 4))

        # ---- constants / weights (loaded once) ----
        wcat = const.tile([C, 4 * C], bf16, tag="wcat", name="wcat")
        nc.sync.dma_start(wcat[:], wcat_d)
        wq_u, wk_u, wv_u, wp = (wcat[:, i * C:(i + 1) * C] for i in range(4))
        bcat = const.tile([C, 3], f32, tag="bcat", name="bcat")
        nc.sync.dma_start(bcat[:], bcat_d)
        bq_t, bv_t, bp_t = (bcat[:, i:i + 1] for i in range(3))
        ones_col = const.tile([C, 1], bf16, tag="ones", name="ones_col")
        nc.vector.memset(ones_col[:], 1.0)

        def emit_compute():
            # ---- big SBUF residents ----
            y_sb = big.tile([C, NQ], bf16, tag="y_sb", name="y_sb")
            nc.sync.dma_start(y_sb[:], yb_d)
            xb_sb = big.tile([C, N], bf16, tag="xb_sb", name="xb_sb")
            for pc in range(6):
                w = N // 6
                nc.sync.dma_start(xb_sb[:, pc * w:(pc + 1) * w],
                                  xb_d[:, pc * w:(pc + 1) * w])
            x32_sb = big.tile([C, NQ], f32, tag="x32_sb", name="x32_sb")
            nc.sync.dma_start(x32_sb[:], x32_d)
            xbT_sb = big.tile([C, N], bf16, tag="xbT_sb", name="xbT_sb")
            for pc in range(3):
                w = N // 3
                nc.sync.dma_start(xbT_sb[:, pc * w:(pc + 1) * w],
                                  xbT_d[:, pc * w:(pc + 1) * w])
            qk_sb = big.tile([C, NQ], bf16, tag="qk_sb", name="qk_sb")
            # softmax-denominator accumulators: main (q cols 0:1024) on DVE
            # with 1-in-8 chunks on gpsimd; the batched tails separately.
            acc = big.tile([C, 1024], bf16, tag="acc", name="acc")
            nc.vector.memset(acc[:], 0.0)
            acc2 = big.tile([C, 1024], bf16, tag="acc2", name="acc2")
            nc.gpsimd.memset(acc2[:], 0.0)
            acc_t = big.tile([C, 512], bf16, tag="acc_t", name="acc_t")
            nc.vector.memset(acc_t[:], 0.0)

            # persistent S^T psum tensors: sA/sB (128,1024) ping-pong [2+2
            # banks] for q cols 0:1024, sT (128,512) [1 bank] stages the last
            # 128 q-columns of four consecutive chunks for one batched exp.
            # RIGHT side so the early release is independent of the left-side
            # pj2 -> po -> ptail stack.
            psS = tc.alloc_tile_pool(name="psS", bufs=1, space="PSUM",
                                     side="right")
            sA = psS.tile([C, 1024], f32, tag="sA", name="sA")
            sB = psS.tile([C, 1024], f32, tag="sB", name="sB")
            sT = psS.tile([C, 512], f32, tag="sT", name="sT")

            # g = Wp x + x + (Wp bv + bp), emitted mid-phase-1 through pj2
            gtiles = {}

            def emit_g_term(pool):
                xq_bf = big.tile([C, NQ], bf16, tag="xq_bf", name="xq_bf")
                nc.vector.tensor_copy(xq_bf[:], x32_sb[:])
                bv_bf = big.tile([C, 1], bf16, tag="bv_bf", name="bv_bf")
                nc.vector.tensor_copy(bv_bf[:], bv_t[:])
                gb_ps = pool.tile([C, 512], f32, tag="pj2", name="gb_ps")
                nc.tensor.matmul(gb_ps[:, :1], wp[:], bv_bf[:], start=True, stop=True)
                gb = big.tile([C, 1], f32, tag="gb", name="gb")
                nc.vector.tensor_scalar_add(gb[:], gb_ps[:, :1], bp_t[:])
                g = big.tile([C, NQ], f32, tag="g", name="g")
                for (c0, w) in Q3:
                    gp = pool.tile([C, 512], f32, tag="pj2", name=f"gp_{c0}")
                    nc.tensor.matmul(gp[:, :w], wp[:], xq_bf[:, c0:c0 + w],
                                     start=True, stop=True)
                    nc.vector.scalar_tensor_tensor(
                        g[:, c0:c0 + w], gp[:, :w], gb[:],
                        x32_sb[:, c0:c0 + w], op0=ADD, op1=ADD)
                gtiles["g"] = g

            pt_tiles = {}
            ptt_tiles = {}

            def emit_s_exp(j):
                """S^T matmuls + exp + denominator accumulation for chunk j.
                q cols 0:1024 take the wide path through sA/sB; the last 128
                columns stage in sT and exp once per 4 chunks."""
                xch = xb_sb[:, j * 128:(j + 1) * 128]
                pt = ptp.tile([C, 1024], bf16, tag="pt", name=f"pt_{j}")
                pt_tiles[j] = pt
                slot = sA if j % 2 == 0 else sB
                nc.tensor.matmul(slot[:, 0:512], xch, qk_sb[:, 0:512],
                                 start=True, stop=True)
                nc.tensor.matmul(slot[:, 512:1024], xch, qk_sb[:, 512:1024],
                                 start=True, stop=True)
                r = j % 4
                nc.tensor.matmul(sT[:, r * 128:(r + 1) * 128], xch,
                                 qk_sb[:, 1024:1152],
                                 start=(r == 0), stop=(r == 3),
                                 skip_group_check=True)
                nc.scalar.activation(pt[:, :1024], slot[:, :1024], AF.Exp,
                                     scale=SCALE)
                if j % 8 == 3:
                    nc.gpsimd.tensor_add(acc2[:], acc2[:], pt[:])
                else:
                    nc.vector.tensor_add(acc[:], acc[:], pt[:])
                if r == 3:
                    g = j // 4
                    ptt = ptt_tiles[g] = ptp.tile([C, 512], bf16, tag="ptt",
                                                  name=f"ptt_{g}", bufs=6)
                    nc.scalar.activation(ptt[:, :512], sT[:, :512], AF.Exp,
                                         scale=SCALE)
                    nc.vector.tensor_add(acc_t[:], acc_t[:], ptt[:])

            def emit_o(j, o_acc):
                """Deferred-able stage-2 accumulation for chunk j (needs the
                chunk's 4-group tail exp done, i.e. chunk 4*(j//4)+3)."""
                xtch = xbT_sb[:, j * 128:(j + 1) * 128]
                pt = pt_tiles.pop(j)
                nc.tensor.matmul(o_acc[:, 0:512], xtch, pt[:, 0:512],
                                 start=(j == 0), stop=(j == NCH - 1),
                                 skip_group_check=True)
                nc.tensor.matmul(o_acc[:, 512:1024], xtch, pt[:, 512:1024],
                                 start=(j == 0), stop=(j == NCH - 1),
                                 skip_group_check=True)
                g, r = j // 4, j % 4
                ptt = ptt_tiles[g]
                nc.tensor.matmul(o_acc[:, 1024:1152], xtch,
                                 ptt[:, r * 128:(r + 1) * 128],
                                 start=(j == 0), stop=(j == NCH - 1),
                                 skip_group_check=True)

            # ---- phase 1: fused-weight prologue (PSUM pool in o_acc's
            # future banks) interleaved with the first CATCH chunks ----
            pj2 = tc.alloc_tile_pool(name="pj2", bufs=3, space="PSUM")
            # WqkT = Wq^T Wk  (so qk = WqkT.T y = (Wk^T Wq) y);  bqk = Wk^T bq
            wqkT = big.tile([C, C], bf16, tag="wqkT", name="wqkT")
            t0p = pj2.tile([C, 512], f32, tag="pj2", name="t0p")
            nc.tensor.matmul(t0p[:, :C], wq_u[:], wk_u[:], start=True, stop=True)
            nc.vector.tensor_copy(wqkT[:], t0p[:, :C])
            bq_bf = big.tile([C, 1], bf16, tag="bq_bf", name="bq_bf")
            nc.vector.tensor_copy(bq_bf[:], bq_t[:])
            t1p = pj2.tile([C, 512], f32, tag="pj2", name="t1p")
            nc.tensor.matmul(t1p[:, :1], wk_u[:], bq_bf[:], start=True, stop=True)
            bqk = big.tile([C, 1], f32, tag="bqk", name="bqk")
            nc.vector.tensor_copy(bqk[:], t1p[:, :1])
            # WfT = (Wp Wv)^T = Wv^T WpT  (output projection of the M path)
            wfT = big.tile([C, C], bf16, tag="wfT", name="wfT")
            t2p = pj2.tile([C, 512], f32, tag="pj2", name="t2p")
            nc.tensor.matmul(t2p[:, :C], wv_u[:], wp[:], start=True, stop=True)
            nc.vector.tensor_copy(wfT[:], t2p[:, :C])
            # qk projection (the only per-token prologue GEMM)
            for (c0, w) in Q3:
                qp = pj2.tile([C, 512], f32, tag="pj2", name=f"qp_{c0}")
                nc.tensor.matmul(qp[:, :w], wqkT[:], y_sb[:, c0:c0 + w],
                                 start=True, stop=True)
                nc.scalar.activation(qk_sb[:, c0:c0 + w], qp[:, :w],
                                     AF.Identity, bias=bqk[:])
            for j in range(CATCH):
                if j == 1:
                    emit_g_term(pj2)
                emit_s_exp(j)
            pj2.release()

            # ---- phase 2: o_acc opens in the freed banks; catch up at two
            # deferred O-chunks per new chunk, then run 1:1 ----
            po = tc.alloc_tile_pool(name="po", bufs=1, space="PSUM")
            o_acc = po.tile([C, NQ], f32, tag="o_acc", name="o_acc")
            next_o = 0
            for j in range(CATCH, NCH):
                emit_s_exp(j)
                budget = 2
                while budget > 0 and next_o <= j - 3:
                    emit_o(next_o, o_acc)
                    next_o += 1
                    budget -= 1
            while next_o < NCH:
                emit_o(next_o, o_acc)
                next_o += 1

            # ---- tail:  out = (Wp O)/r + g.
            # sA/sB/sT are dead now; their banks host the tail psum pool. ----
            psS.release()
            ptail = tc.alloc_tile_pool(name="ptail", bufs=1, space="PSUM")
            g = gtiles["g"]
            o_bf = big.tile([C, NQ], bf16, tag="o_bf", name="o_bf")
            rs_row = big.tile([1, NQ], f32, tag="rs_row", name="rs_row")
            recip = big.tile([1, NQ], f32, tag="recip", name="recip")
            rb = big.tile([C, NQ], f32, tag="rb", name="rb")
            t2 = big.tile([C, NQ], f32, tag="t2", name="t2")
            out_sb = big.tile([C, NQ], f32, tag="out_sb", name="out_sb")
            # denominators: main part from acc+acc2, tails folded from acc_t
            rpA = ptail.tile([C, 1024], f32, tag="rp", name="rpA")
            for c0 in (0, 512):
                nc.tensor.matmul(rpA[:1, c0:c0 + 512], ones_col[:],
                                 acc[:, c0:c0 + 512], start=True, stop=False)
                nc.tensor.matmul(rpA[:1, c0:c0 + 512], ones_col[:],
                                 acc2[:, c0:c0 + 512], start=False, stop=True)
            nc.vector.tensor_copy(rs_row[:, 0:1024], rpA[:1, :1024])
            rpB = ptail.tile([C, 512], f32, tag="rpt", name="rpB")
            nc.tensor.matmul(rpB[:1, :512], ones_col[:], acc_t[:, :512],
                             start=True, stop=True)
            tsb = big.tile([1, 512], f32, tag="tsb", name="tsb")
            nc.vector.tensor_copy(tsb[:], rpB[:1, :512])
            nc.vector.tensor_add(rs_row[:, 1024:1152], tsb[:, 0:128],
                                 tsb[:, 128:256])
            nc.vector.tensor_add(rs_row[:, 1024:1152], rs_row[:, 1024:1152],
                                 tsb[:, 256:384])
            nc.vector.tensor_add(rs_row[:, 1024:1152], rs_row[:, 1024:1152],
                                 tsb[:, 384:512])
            nc.vector.reciprocal_approx_fast(out=recip[:], in_=rs_row[:])
            nc.gpsimd.partition_broadcast(rb[:], recip[:])
            # O evac + projection + normalize, per q-half
            for h0 in (0, HALF):
                nc.vector.tensor_copy(o_bf[:, h0:h0 + HALF],
                                      o_acc[:, h0:h0 + HALF])
                pw = ptail.tile([C, HALF], f32, tag="pw", name=f"pw_{h0}")
                nc.tensor.matmul(pw[:, :512], wfT[:], o_bf[:, h0:h0 + 512],
                                 start=True, stop=True)
                nc.tensor.matmul(pw[:, 512:HALF], wfT[:],
                                 o_bf[:, h0 + 512:h0 + HALF],
                                 start=True, stop=True)
                nc.vector.tensor_mul(t2[:, h0:h0 + HALF], pw[:, :HALF],
                                     rb[:, h0:h0 + HALF])
                nc.vector.tensor_add(out_sb[:, h0:h0 + HALF],
                                     t2[:, h0:h0 + HALF], g[:, h0:h0 + HALF])
                nc.sync.dma_start(out_d[:, h0:h0 + HALF],
                                  out_sb[:, h0:h0 + HALF])
            ptail.release()
            po.release()

        for _rep in range(repeat):
            emit_compute()

    nc.compile()
    return nc


def make_in_maps(x, y, Wq, bq, Wk, bk, Wv, bv, Wp, bp):
    """Host-side sharding: slice q/residual tokens per core, cast matmul
    operands to bf16, pre-transpose the 1x1-conv weights into lhsT layout."""
    x2 = np.asarray(x, np.float32).reshape(C, N)
    y2 = np.asarray(y, np.float32).reshape(C, N)
    xb = np.ascontiguousarray(x2).astype(BF16)
    # per-chunk transposed x: xbT[p, ch*128 + c] = x2[c, ch*128 + p]
    xbT = np.ascontiguousarray(
        x2.reshape(C, NCH, 128).transpose(2, 1, 0).reshape(128, N)).astype(BF16)
    # Wq/Wk/Wv untransposed (fused on device), Wp pre-transposed
    wcat = np.ascontiguousarray(np.concatenate(
        [np.asarray(Wq, np.float32), np.asarray(Wk, np.float32),
         np.asarray(Wv, np.float32), np.asarray(Wp, np.float32).T],
        axis=1)).astype(BF16)
    bcat = np.ascontiguousarray(np.stack(
        [np.asarray(b, np.float32) for b in (bq, bv, bp)], axis=1))
    in_maps = []
    for i in range(NCORES):
        sl = slice(i * NQ, (i + 1) * NQ)
        in_maps.append({
            "xb": xb, "xbT": xbT,
            "x32": np.ascontiguousarray(x2[:, sl]),
            "yb": np.ascontiguousarray(y2[:, sl]).astype(BF16),
            "wcat": wcat, "bcat": bcat,
        })
    return in_maps


_CACHE: dict = {}


class Runner:
    """Compiles the SPMD program once and exposes a repeat-callable runner
    (mirrors concourse.bass2jax.run_bass_via_pjrt's multi-core path, but
    caches the jitted executable so repeat calls don't recompile)."""

    def __init__(self, repeat: int = 1):
        import jax
        try:
            jax.config.update("jax_compilation_cache_dir", "/tmp/jax_neff_cache")
            jax.config.update("jax_persistent_cache_min_compile_time_secs", 1.0)
        except Exception:
            pass
        from jax.sharding import Mesh, PartitionSpec, NamedSharding
        from jax.experimental.shard_map import shard_map
        from concourse import mybir
        from concourse import bass2jax

        bass2jax.install_neuronx_cc_hook()
        nc = _build_nc(repeat=repeat)
        self.nc = nc
        self.jax = jax

        partition_name = nc.partition_id_tensor.name if nc.partition_id_tensor else None
        in_names, out_names, out_avals, zero_templates = [], [], [], []
        for alloc in nc.m.functions[0].allocations:
            if not isinstance(alloc, mybir.MemoryLocationSet):
                continue
            name = alloc.memorylocations[0].name
            if alloc.kind == "ExternalInput":
                if name != partition_name:
                    in_names.append(name)
            elif alloc.kind == "ExternalOutput":
                out_names.append(name)
                shape = tuple(alloc.tensor_shape)
                dtype = mybir.dt.np(alloc.dtype)
                out_avals.append(jax.core.ShapedArray(shape, dtype))
                zero_templates.append(np.zeros(shape, dtype))
        self.in_names, self.out_names = in_names, out_names
        self.out_avals, self.zero_templates = out_avals, zero_templates
        n_params = len(in_names)
        self.n_params = n_params
        all_in_names = tuple(in_names) + tuple(out_names)
        if partition_name is not None:
            all_in_names = all_in_names + (partition_name,)

        def _body(*args):
            operands = list(args)
            if partition_name is not None:
                operands.append(bass2jax.partition_id_tensor())
            outs = bass2jax._bass_exec_p.bind(
                *operands,
                out_avals=tuple(out_avals),
                in_names=all_in_names,
                out_names=tuple(out_names),
                lowering_input_output_aliases=(),
                sim_require_finite=True,
                sim_require_nnan=True,
                nc=nc,
            )
            return tuple(outs)

        devices = jax.devices()[:NCORES]
        assert len(devices) == NCORES, f"need {NCORES} cores, got {len(devices)}"
        self.mesh = Mesh(np.asarray(devices), ("core",))
        self.spec = PartitionSpec("core")
        self.sharding = NamedSharding(self.mesh, self.spec)
        n_outs = len(out_names)
        in_specs = (self.spec,) * (n_params + n_outs)
        out_specs = (self.spec,) * n_outs
        # no donation: lets us reuse staged device buffers across timed calls
        self.sharded = jax.jit(
            shard_map(_body, mesh=self.mesh, in_specs=in_specs,
                      out_specs=out_specs, check_rep=False),
            keep_unused=True,
        )

    def stage(self, in_maps):
        """device_put the concatenated per-core inputs (+ zero out-buffers)."""
        jax = self.jax
        concat = [
            np.concatenate([np.asarray(in_maps[c][nm]) for c in range(NCORES)], axis=0)
            for nm in self.in_names
        ]
        concat += [
            np.zeros((NCORES * z.shape[0],) + z.shape[1:], z.dtype)
            for z in self.zero_templates
        ]
        return [jax.device_put(a, self.sharding) for a in concat]

    def run_staged(self, staged):
        return self.sharded(*staged)

    def __call__(self, in_maps):
        jax = self.jax
        out_arrs = self.sharded(*self.stage(in_maps))
        out_arrs = [np.asarray(a) for a in jax.block_until_ready(out_arrs)]
        results = []
        for c in range(NCORES):
            results.append({
                nm: out_arrs[i].reshape(NCORES, *self.out_avals[i].shape)[c]
                for i, nm in enumerate(self.out_names)
            })
        return results


def get_runner(repeat: int = 1):
    key = ("runner", repeat)
    if key not in _CACHE:
        _CACHE[key] = Runner(repeat=repeat)
    return _CACHE[key]


def kernel(**inputs) -> np.ndarray:
    runner = get_runner()
    in_maps = make_in_maps(**{k: inputs[k] for k in
                              ("x", "y", "Wq", "bq", "Wk", "bk", "Wv", "bv", "Wp", "bp")})
    results = runner(in_maps)
    out = np.concatenate([results[i]["out"] for i in range(NCORES)], axis=1)
    return out.reshape(1, C, Z, HH, WW).astype(np.float32)
